# revision 24
# baseline (speedup 1.0000x reference)
"""GAT layer kernel for Trainium2, data-parallel over batch across 8 NeuronCores.

Per batch element b (one core each):
    hp  = h @ W_proj + b_proj                      # [N, D]
    s   = hp @ w_src ; t = hp @ w_dst              # [N]
    e   = relu(s[:,None] + t[None,:] + b_att)      # [N, N]
    att = exp(e) * a ; att /= att.sum(-1, keepdim) # [N, N]
    out = att @ hp + hp                            # [N, D]

Identities:
  exp(relu(x)) == max(exp(x), 1)
  exp(s_i+t_j+b) == u_i * v_j with u = exp(s), v = exp(t + b)

v3 design (vs the 89us v2):
  - a is the only big tensor (16.8 MB f32 read = ~40us at the ~420 GB/s
    per-core DMA ceiling); the kernel must be a-load-bound.  v2 issued the
    a-load ~11us late (behind gpsimd setup) and at 4MB-quad granularity,
    then had a 24us un-pipelined compute tail.  v3:
      * the 16 per-row-block cast-DMAs (f32->bf16) are the FIRST
        instructions on the gpsimd/SWDGE ring; nothing precedes them.
      * identity comes in as a host-staged input (no on-chip iota), so
        gpsimd only does DMA descriptor gen + the residual adds.
      * per block: z=max(v*u_r,1) (DVE 4x) -> pb=z*a_r (DVE 2x) ->
        16 PE transposes in 2 PSUM groups -> PSUM->SBUF copies split
        ACT(1536 cols)/DVE(512 cols) -> 16 accumulating matmuls whose
        rhs hp_aug carries a ones column so the rowsum lands in psum
        col 128 -> lagged finalize (recip, scale, +hp residual, store).
      * steady-state per-block engine budgets (~2.2-2.5us) all sit under
        the 2.63us/block DMA arrival cadence.
"""

import os
import sys

for _p in ("/opt/trn_rl_repo", "/root/.axon_site/_ro/trn_rl_repo"):
    if _p not in sys.path and os.path.isdir(_p):
        sys.path.append(_p)

import numpy as np
from contextlib import ExitStack

import concourse.bass as bass
import concourse.bacc as bacc
import concourse.tile as tile
from concourse import mybir
from concourse.bass_utils import run_bass_kernel_spmd

F32 = mybir.dt.float32
BF16 = mybir.dt.bfloat16

B, N, D = 8, 2048, 128
P = 128           # partitions
NT = N // P       # 16 row/col blocks
N_CORES = 8

AF = mybir.ActivationFunctionType
ALU = mybir.AluOpType


def _build_kernel(ctx: ExitStack, tc: tile.TileContext, io: dict):
    nc = tc.nc
    a = io["a"]            # [N, N] f32 dram
    h = io["h"]            # [N, D] f32 dram
    params_in = io["params"]  # [P, 260] f32: [W | ident | bp | ws | wd | ba]
    out = io["out"]        # [N, D] f32 dram

    cst = ctx.enter_context(tc.tile_pool(name="cst", bufs=1))
    sps = ctx.enter_context(tc.tile_pool(name="sps", bufs=1, space="PSUM"))
    a_pool = ctx.enter_context(tc.tile_pool(name="a", bufs=1))

    # DMA facts (measured): no packet moves before ~8.2us (fixed engine
    # init); the SWDGE (gpsimd) ring lands its first packets ~10us; once
    # the SWDGE a-stream runs, concurrent HWDGE traffic is starved to
    # ~1/8 rate and >8 queued HWDGE DMAs block the issuing engine.  So the
    # HWDGE ring carries ONLY what fits the 8.2-10us gap (params + h, one
    # DMA each), and all 16 a-blocks stream on SWDGE, per-block so the
    # main loop starts on block 0 at ~13us and tracks the 2.5us/block
    # arrival cadence.
    a_tiles = {}
    # singles first (fast pipeline start), quads in the middle (best DMA
    # efficiency while stream-bound), singles at the end (short tail).
    # hT rides as the second transfer: arrives ~12us, no HWDGE race.
    chunks = [(0, 1), ("hT", 0), (1, 1), (2, 1), (3, 1),
              (4, 4), (8, 4), (12, 2), (14, 1), (15, 1)]
    hT_t = cst.tile([P, N], BF16)
    for s, u in chunks:
        if s == "hT":
            nc.gpsimd.dma_start(hT_t[:], h[:])
            continue
        a_t = a_pool.tile([P, u, N], BF16, tag=f"a{s}")
        nc.gpsimd.dma_start(
            a_t[:], a[s * P:(s + u) * P, :].rearrange("(u p) j -> p u j", p=P))
        for k in range(u):
            a_tiles[s + k] = a_t[:, k, :]

    # ---- h then packed params on the HWDGE ring: ONLY 2 DMAs, issued
    # before the SWDGE descriptors hit the queues (FIFO per queue -> they
    # drain first).  h loads contiguous (8KB descriptors): h_lin[p, c*D+d]
    # = h[16p+c, d]; the hT copies un-permute with a stride-16 scatter. ----
    prm = cst.tile([P, 261], F32)
    nc.sync.dma_start(prm[:], params_in[:])
    W_sb = prm[:, 0:D]
    ident_f = prm[:, D:2 * D]
    bp_col = prm[:, 256:257]
    wsd_sb = prm[:, 257:259]   # [W@w_src | W@w_dst], host-folded
    bs_col = prm[:, 259:260]   # b_proj . w_src
    bd_row = prm[:1, 260:261]  # b_proj . w_dst + b_att

    ident = cst.tile([P, P], BF16)
    nc.vector.tensor_copy(ident[:], ident_f)
    W_b16 = cst.tile([P, D], BF16)
    nc.vector.tensor_copy(W_b16[:], W_sb)
    wsd_b16 = cst.tile([P, 2], BF16)
    nc.vector.tensor_copy(wsd_b16[:], wsd_sb)
    ones_c = cst.tile([1, P], BF16)
    nc.vector.memset(ones_c[:], 1.0)

    # ---- h arrives pre-transposed (hT [d, n], host layout transform).
    # u and v come straight from hT with host-folded weight vectors
    # (s = h @ (W w_src) + b.w_src etc), so z_0 is not gated by hpT. ----
    u_sb = cst.tile([P, NT], F32)
    s_ps = sps.tile([P, 512], F32, tag="spu")
    for r in range(NT):
        nc.tensor.matmul(s_ps[:, r:r + 1], hT_t[:, r * P:(r + 1) * P],
                         wsd_b16[:, 0:1])
    nc.scalar.activation(u_sb[:], s_ps[:, :NT], AF.Exp, bias=bs_col)

    v_row = cst.tile([1, N], BF16)
    v_full = cst.tile([P, N], BF16)
    for g in range(4):
        sl = slice(g * 512, (g + 1) * 512)
        vp = sps.tile([P, 512], F32, tag=f"sp{g % 2}")
        nc.tensor.matmul(vp[:1, :], wsd_b16[:, 1:2], hT_t[:, sl])
        nc.scalar.activation(v_row[:, sl], vp[:1, :], AF.Exp,
                             bias=bd_row, scale=1.0)
        ps = sps.tile([P, 512], F32, tag=f"sp{g % 2}")
        nc.tensor.matmul(ps[:], ones_c[:], v_row[:, sl])
        nc.scalar.copy(v_full[:, sl], ps[:])

    # ---- hpT = (h @ W + b).T, then hp natural via PE transposes ----
    hpT = cst.tile([P, N], BF16)
    hp_aug = cst.tile([P, NT, 130], BF16)
    nc.vector.memset(hp_aug[:, :, D:D + 1], 1.0)
    for g in range(4):
        sl = slice(g * 512, (g + 1) * 512)
        ps = sps.tile([P, 512], F32, tag=f"sp{g % 2}")
        nc.tensor.matmul(ps[:], W_b16[:], hT_t[:, sl])
        nc.scalar.activation(hpT[:, sl], ps[:], AF.Identity,
                             bias=bp_col, scale=1.0)
        tp = sps.tile([P, 512], BF16, tag=f"sp{g % 2}")
        for c4 in range(4):
            r = 4 * g + c4
            nc.tensor.matmul(tp[:, c4 * P:(c4 + 1) * P],
                             hpT[:, r * P:(r + 1) * P], ident[:],
                             is_transpose=True)
        nc.scalar.copy(
            hp_aug[:, 4 * g:4 * g + 4, 0:D].bitcast(F32),
            tp[:].rearrange("p (a b) -> p a b", a=4).bitcast(F32))

    # ---- main loop pools ----
    z_pool = ctx.enter_context(tc.tile_pool(name="z", bufs=1))
    pb_pool = ctx.enter_context(tc.tile_pool(name="pb", bufs=1))
    pbt_pool = ctx.enter_context(tc.tile_pool(name="pbt", bufs=1))
    tps_pool = ctx.enter_context(tc.tile_pool(name="tps", bufs=1, space="PSUM"))
    ops_pool = ctx.enter_context(tc.tile_pool(name="ops", bufs=1, space="PSUM"))
    rs_pool = ctx.enter_context(tc.tile_pool(name="rs", bufs=1))
    osb_pool = ctx.enter_context(tc.tile_pool(name="osb", bufs=1))
    o2_pool = ctx.enter_context(tc.tile_pool(name="o2", bufs=1))

    out_t = out.rearrange("(r p) d -> p r d", p=P)

    # finalize lagged 2 blocks so in-order ACT/DVE streams never stall on
    # the current block's matmul group
    pending = []

    def finalize(o_ps, r):
        rinv = rs_pool.tile([P, 1], F32, tag=f"ri{r % 2}")
        nc.vector.reciprocal(rinv[:], o_ps[:, D:D + 1])
        o_sb = osb_pool.tile([P, D], F32, tag=f"os{r % 3}")
        nc.scalar.activation(o_sb[:], o_ps[:, 0:D], AF.Copy, scale=rinv[:])
        o2 = o2_pool.tile([P, D], BF16, tag=f"o2{r % 3}")
        nc.gpsimd.tensor_tensor(o2[:], o_sb[:], hp_aug[:, r, 0:D], ALU.add)
        nc.sync.dma_start(out_t[:, r, :], o2[:])

    for r in range(NT):
        z_t = z_pool.tile([P, N], BF16, tag=f"z{r % 4}")
        nc.vector.tensor_scalar(z_t[:], v_full[:], u_sb[:, r:r + 1], 1.0,
                                ALU.mult, ALU.max)
        pb = pb_pool.tile([P, N], BF16, tag=f"pb{r % 4}")
        nc.vector.tensor_tensor(pb[:], z_t[:], a_tiles[r], ALU.mult)

        pbT = pbt_pool.tile([P, N], BF16, tag=f"pt{r % 4}")
        o_ps = ops_pool.tile([P, 132], F32, tag=f"o{r % 2}")
        for g in range(2):
            tp = tps_pool.tile([P, 8 * P], BF16, tag=f"tp{(2 * r + g) % 3}")
            for c8 in range(8):
                c = 8 * g + c8
                nc.tensor.matmul(tp[:, c8 * P:(c8 + 1) * P],
                                 pb[:, c * P:(c + 1) * P], ident[:],
                                 is_transpose=True)
            # PSUM->SBUF copies: all on ACT, bitcast to f32 so the
            # per-element engine cost covers two bf16 values per cycle
            nc.scalar.copy(pbT[:, g * 1024:(g + 1) * 1024].bitcast(F32),
                           tp[:].bitcast(F32))
            for c8 in range(8):
                c = 8 * g + c8
                nc.tensor.matmul(o_ps[:, 0:D + 1], pbT[:, c * P:(c + 1) * P],
                                 hp_aug[:, c, 0:D + 1],
                                 start=(c == 0), stop=(c == NT - 1))

        pending.append((o_ps, r))
        if len(pending) > 2:
            finalize(*pending.pop(0))

    for item in pending:
        finalize(*item)


_CACHE = {}


def _get_compiled():
    if "nc" in _CACHE:
        return _CACHE["nc"], _CACHE["names"]

    nc = bacc.Bacc("TRN2", target_bir_lowering=False, debug=False)
    io = {}
    io["a"] = nc.dram_tensor("a", [N, N], F32, kind="ExternalInput").ap()
    io["h"] = nc.dram_tensor("h", [D, N], F32, kind="ExternalInput").ap()
    io["params"] = nc.dram_tensor("params", [P, 261], F32, kind="ExternalInput").ap()
    io["out"] = nc.dram_tensor("out", [N, D], BF16, kind="ExternalOutput").ap()

    with tile.TileContext(nc) as tc:
        with ExitStack() as ctx:
            _build_kernel(ctx, tc, io)
    nc.compile()

    _CACHE["nc"] = nc
    _CACHE["names"] = list(io.keys())
    return nc, _CACHE["names"]


def _make_in_maps(a, h, W_proj, b_proj, w_att, b_att):
    a = np.ascontiguousarray(a, dtype=np.float32)
    h = np.ascontiguousarray(h, dtype=np.float32)
    W_proj = np.ascontiguousarray(W_proj, dtype=np.float32)
    b_proj = np.ascontiguousarray(b_proj, dtype=np.float32).reshape(D, 1)
    w_att = np.ascontiguousarray(w_att, dtype=np.float32)
    w_sd = np.stack([w_att[:D], w_att[D:]], axis=1).copy()  # [D, 2]
    b_att = np.asarray(b_att, dtype=np.float32).reshape(1, 1).copy()

    params = np.zeros((P, 261), dtype=np.float32)
    params[:, 0:D] = W_proj
    params[:, D:2 * D] = np.eye(P, dtype=np.float32)
    params[:, 256:257] = b_proj
    params[:, 257] = W_proj @ w_att[:D]
    params[:, 258] = W_proj @ w_att[D:]
    params[:, 259] = float(b_proj[:, 0] @ w_att[:D])
    params[0, 260] = float(b_proj[:, 0] @ w_att[D:]) + b_att[0, 0]

    in_maps = []
    for c in range(N_CORES):
        in_maps.append({"a": a[c],
                        "h": np.ascontiguousarray(h[c].T),
                        "params": params})
    return in_maps


def _get_executable():
    """Build (once) a sharded PJRT callable for the compiled Bass module.

    Mirrors concourse.bass2jax.run_bass_via_pjrt but keeps the jitted
    function so repeated calls don't retrace/recompile.
    """
    if "exe" in _CACHE:
        return _CACHE["exe"]

    import jax
    from jax.sharding import Mesh, PartitionSpec
    from jax.experimental.shard_map import shard_map
    from concourse import bass2jax, mybir as _mybir

    nc, _ = _get_compiled()
    bass2jax.install_neuronx_cc_hook()

    partition_name = (nc.partition_id_tensor.name
                      if nc.partition_id_tensor else None)
    in_names, out_names, out_avals, zero_outs = [], [], [], []
    for alloc in nc.m.functions[0].allocations:
        if not isinstance(alloc, _mybir.MemoryLocationSet):
            continue
        name = alloc.memorylocations[0].name
        if alloc.kind == "ExternalInput":
            if name != partition_name:
                in_names.append(name)
        elif alloc.kind == "ExternalOutput":
            shape = tuple(alloc.tensor_shape)
            dtype = _mybir.dt.np(alloc.dtype)
            out_names.append(name)
            out_avals.append(jax.core.ShapedArray(shape, dtype))
            zero_outs.append(np.zeros(shape, dtype))
    n_params = len(in_names)
    n_outs = len(out_avals)
    all_in_names = in_names + out_names + (
        [partition_name] if partition_name else [])
    donate = tuple(range(n_params, n_params + n_outs))

    def _body(*args):
        operands = list(args)
        if partition_name is not None:
            operands.append(bass2jax.partition_id_tensor())
        outs = bass2jax._bass_exec_p.bind(
            *operands,
            out_avals=tuple(out_avals),
            in_names=tuple(all_in_names),
            out_names=tuple(out_names),
            lowering_input_output_aliases=(),
            sim_require_finite=True,
            sim_require_nnan=True,
            nc=nc,
        )
        return tuple(outs)

    devices = jax.devices()[:N_CORES]
    mesh = Mesh(np.asarray(devices), ("core",))
    in_specs = (PartitionSpec("core"),) * (n_params + n_outs)
    out_specs = (PartitionSpec("core"),) * n_outs
    fn = jax.jit(
        shard_map(_body, mesh=mesh, in_specs=in_specs, out_specs=out_specs,
                  check_rep=False),
        donate_argnums=donate, keep_unused=True,
    )
    exe = {
        "fn": fn, "mesh": mesh, "in_names": in_names,
        "out_names": out_names, "out_avals": out_avals,
        "zero_outs": zero_outs, "n_params": n_params,
    }
    _CACHE["exe"] = exe
    return exe


def _concat_inputs(exe, in_maps):
    return [
        np.concatenate([np.asarray(in_maps[c][name])
                        for c in range(N_CORES)], axis=0)
        for name in exe["in_names"]
    ]


def _concat_zeros(exe):
    return [np.zeros((N_CORES * z.shape[0], *z.shape[1:]), z.dtype)
            for z in exe["zero_outs"]]


def kernel(a, h, W_proj, b_proj, w_att, b_att):
    exe = _get_executable()
    in_maps = _make_in_maps(a, h, W_proj, b_proj, w_att, b_att)
    out_arrs = exe["fn"](*_concat_inputs(exe, in_maps), *_concat_zeros(exe))
    i = exe["out_names"].index("out")
    return np.asarray(out_arrs[i]).astype(np.float32).reshape(N_CORES, N, D)


if __name__ == "__main__":
    rng = np.random.default_rng(0)
    a = rng.random((B, N, N), dtype=np.float32)
    h = rng.standard_normal((B, N, D)).astype(np.float32)
    W_proj = (rng.standard_normal((D, D)) / np.sqrt(D)).astype(np.float32)
    b_proj = (rng.standard_normal(D) * 0.01).astype(np.float32)
    w_att = (rng.standard_normal(2 * D) / np.sqrt(2 * D)).astype(np.float32)
    b_att = np.float32(rng.standard_normal() * 0.01)

    got = kernel(a=a, h=h, W_proj=W_proj, b_proj=b_proj, w_att=w_att,
                 b_att=b_att)

    hp = h @ W_proj + b_proj
    s = hp @ w_att[:D]
    t = hp @ w_att[D:]
    e = np.maximum(s[:, :, None] + t[:, None, :] + b_att, 0.0)
    att = np.exp(e) * a
    att = att / att.sum(-1, keepdims=True)
    ref = att @ hp + hp

    err = np.abs(got - ref).max() / np.abs(ref).max()
    print("rel err:", err)


# revision 25
# speedup vs baseline: 1.1329x; 1.1329x over previous
"""GAT layer kernel for Trainium2, data-parallel over batch across 8 NeuronCores.

Per batch element b (one core each):
    hp  = h @ W_proj + b_proj                      # [N, D]
    s   = hp @ w_src ; t = hp @ w_dst              # [N]
    e   = relu(s[:,None] + t[None,:] + b_att)      # [N, N]
    att = exp(e) * a ; att /= att.sum(-1, keepdim) # [N, N]
    out = att @ hp + hp                            # [N, D]

Identities:
  exp(relu(x)) == max(exp(x), 1)
  exp(s_i+t_j+b) == u_i * v_j with u = exp(s), v = exp(t + b)

v3 design (vs the 89us v2):
  - a is the only big tensor (16.8 MB f32 read = ~40us at the ~420 GB/s
    per-core DMA ceiling); the kernel must be a-load-bound.  v2 issued the
    a-load ~11us late (behind gpsimd setup) and at 4MB-quad granularity,
    then had a 24us un-pipelined compute tail.  v3:
      * the 16 per-row-block cast-DMAs (f32->bf16) are the FIRST
        instructions on the gpsimd/SWDGE ring; nothing precedes them.
      * identity comes in as a host-staged input (no on-chip iota), so
        gpsimd only does DMA descriptor gen + the residual adds.
      * per block: z=max(v*u_r,1) (DVE 4x) -> pb=z*a_r (DVE 2x) ->
        16 PE transposes in 2 PSUM groups -> PSUM->SBUF copies split
        ACT(1536 cols)/DVE(512 cols) -> 16 accumulating matmuls whose
        rhs hp_aug carries a ones column so the rowsum lands in psum
        col 128 -> lagged finalize (recip, scale, +hp residual, store).
      * steady-state per-block engine budgets (~2.2-2.5us) all sit under
        the 2.63us/block DMA arrival cadence.
"""

import os
import sys

for _p in ("/opt/trn_rl_repo", "/root/.axon_site/_ro/trn_rl_repo"):
    if _p not in sys.path and os.path.isdir(_p):
        sys.path.append(_p)

import numpy as np
from contextlib import ExitStack

import concourse.bass as bass
import concourse.bacc as bacc
import concourse.tile as tile
from concourse import mybir
from concourse.bass_utils import run_bass_kernel_spmd

F32 = mybir.dt.float32
BF16 = mybir.dt.bfloat16

B, N, D = 8, 2048, 128
P = 128           # partitions
NT = N // P       # 16 row/col blocks
N_CORES = 8

AF = mybir.ActivationFunctionType
ALU = mybir.AluOpType


def _build_kernel(ctx: ExitStack, tc: tile.TileContext, io: dict):
    nc = tc.nc
    a = io["a"]            # [N, N] f32 dram
    h = io["h"]            # [N, D] f32 dram
    params_in = io["params"]  # [P, 260] f32: [W | ident | bp | ws | wd | ba]
    out = io["out"]        # [N, D] f32 dram

    cst = ctx.enter_context(tc.tile_pool(name="cst", bufs=1))
    sps = ctx.enter_context(tc.tile_pool(name="sps", bufs=1, space="PSUM"))
    a_pool = ctx.enter_context(tc.tile_pool(name="a", bufs=1))

    # DMA facts (measured): no packet moves before ~8.2us (fixed engine
    # init); the SWDGE (gpsimd) ring lands its first packets ~10us; once
    # the SWDGE a-stream runs, concurrent HWDGE traffic is starved to
    # ~1/8 rate and >8 queued HWDGE DMAs block the issuing engine.  So the
    # HWDGE ring carries ONLY what fits the 8.2-10us gap (params + h, one
    # DMA each), and all 16 a-blocks stream on SWDGE, per-block so the
    # main loop starts on block 0 at ~13us and tracks the 2.5us/block
    # arrival cadence.
    a_tiles = {}
    # singles first (fast pipeline start), quads in the middle (best DMA
    # efficiency while stream-bound), singles at the end (short tail).
    # hT rides as the second transfer: arrives ~12us, no HWDGE race.
    chunks = [(0, 1), ("hT", 0), (1, 1), (2, 1), (3, 1),
              (4, 2), (6, 2), (8, 2), (10, 2), (12, 2), (14, 1), (15, 1)]
    hT_t = cst.tile([P, N], BF16)
    for s, u in chunks:
        if s == "hT":
            nc.gpsimd.dma_start(hT_t[:], h[:])
            continue
        a_t = a_pool.tile([P, u, N], BF16, tag=f"a{s}")
        nc.gpsimd.dma_start(
            a_t[:], a[s * P:(s + u) * P, :].rearrange("(u p) j -> p u j", p=P))
        for k in range(u):
            a_tiles[s + k] = a_t[:, k, :]

    # ---- h then packed params on the HWDGE ring: ONLY 2 DMAs, issued
    # before the SWDGE descriptors hit the queues (FIFO per queue -> they
    # drain first).  h loads contiguous (8KB descriptors): h_lin[p, c*D+d]
    # = h[16p+c, d]; the hT copies un-permute with a stride-16 scatter. ----
    prm = cst.tile([P, 261], F32)
    nc.sync.dma_start(prm[:], params_in[:])
    W_sb = prm[:, 0:D]
    ident_f = prm[:, D:2 * D]
    bp_col = prm[:, 256:257]
    wsd_sb = prm[:, 257:259]   # [W@w_src | W@w_dst], host-folded
    bs_col = prm[:, 259:260]   # b_proj . w_src
    bd_row = prm[:1, 260:261]  # b_proj . w_dst + b_att

    ident = cst.tile([P, P], BF16)
    nc.vector.tensor_copy(ident[:], ident_f)
    W_b16 = cst.tile([P, D], BF16)
    nc.vector.tensor_copy(W_b16[:], W_sb)
    wsd_b16 = cst.tile([P, 2], BF16)
    nc.vector.tensor_copy(wsd_b16[:], wsd_sb)
    ones_c = cst.tile([1, P], BF16)
    nc.vector.memset(ones_c[:], 1.0)

    # ---- h arrives pre-transposed (hT [d, n], host layout transform).
    # u and v come straight from hT with host-folded weight vectors
    # (s = h @ (W w_src) + b.w_src etc), so z_0 is not gated by hpT. ----
    u_sb = cst.tile([P, NT], F32)
    s_ps = sps.tile([P, 512], F32, tag="spu")
    for r in range(NT):
        nc.tensor.matmul(s_ps[:, r:r + 1], hT_t[:, r * P:(r + 1) * P],
                         wsd_b16[:, 0:1])
    nc.scalar.activation(u_sb[:], s_ps[:, :NT], AF.Exp, bias=bs_col)

    v_row = cst.tile([1, N], BF16)
    v_full = cst.tile([P, N], BF16)
    for g in range(4):
        sl = slice(g * 512, (g + 1) * 512)
        vp = sps.tile([P, 512], F32, tag=f"sp{g % 2}")
        nc.tensor.matmul(vp[:1, :], wsd_b16[:, 1:2], hT_t[:, sl])
        nc.scalar.activation(v_row[:, sl], vp[:1, :], AF.Exp,
                             bias=bd_row, scale=1.0)
        ps = sps.tile([P, 512], F32, tag=f"sp{g % 2}")
        nc.tensor.matmul(ps[:], ones_c[:], v_row[:, sl])
        nc.scalar.copy(v_full[:, sl], ps[:])

    # ---- hpT = (h @ W + b).T, then hp natural via PE transposes ----
    hpT = cst.tile([P, N], BF16)
    hp_aug = cst.tile([P, NT, 130], BF16)
    nc.vector.memset(hp_aug[:, :, D:D + 1], 1.0)
    for g in range(4):
        sl = slice(g * 512, (g + 1) * 512)
        ps = sps.tile([P, 512], F32, tag=f"sp{g % 2}")
        nc.tensor.matmul(ps[:], W_b16[:], hT_t[:, sl])
        nc.scalar.activation(hpT[:, sl], ps[:], AF.Identity,
                             bias=bp_col, scale=1.0)
        tp = sps.tile([P, 512], BF16, tag=f"sp{g % 2}")
        for c4 in range(4):
            r = 4 * g + c4
            nc.tensor.matmul(tp[:, c4 * P:(c4 + 1) * P],
                             hpT[:, r * P:(r + 1) * P], ident[:],
                             is_transpose=True)
        nc.scalar.copy(
            hp_aug[:, 4 * g:4 * g + 4, 0:D].bitcast(F32),
            tp[:].rearrange("p (a b) -> p a b", a=4).bitcast(F32))

    # ---- main loop pools ----
    z_pool = ctx.enter_context(tc.tile_pool(name="z", bufs=1))
    pb_pool = ctx.enter_context(tc.tile_pool(name="pb", bufs=1))
    pbt_pool = ctx.enter_context(tc.tile_pool(name="pbt", bufs=1))
    tps_pool = ctx.enter_context(tc.tile_pool(name="tps", bufs=1, space="PSUM"))
    ops_pool = ctx.enter_context(tc.tile_pool(name="ops", bufs=1, space="PSUM"))
    rs_pool = ctx.enter_context(tc.tile_pool(name="rs", bufs=1))
    osb_pool = ctx.enter_context(tc.tile_pool(name="osb", bufs=1))
    o2_pool = ctx.enter_context(tc.tile_pool(name="o2", bufs=1))

    out_t = out.rearrange("(r p) d -> p r d", p=P)

    # finalize lagged 2 blocks so in-order ACT/DVE streams never stall on
    # the current block's matmul group
    pending = []

    def finalize(o_ps, r):
        rinv = rs_pool.tile([P, 1], F32, tag=f"ri{r % 2}")
        nc.vector.reciprocal(rinv[:], o_ps[:, D:D + 1])
        o_sb = osb_pool.tile([P, D], F32, tag=f"os{r % 3}")
        nc.scalar.activation(o_sb[:], o_ps[:, 0:D], AF.Copy, scale=rinv[:])
        o2 = o2_pool.tile([P, D], BF16, tag=f"o2{r % 3}")
        nc.gpsimd.tensor_tensor(o2[:], o_sb[:], hp_aug[:, r, 0:D], ALU.add)
        nc.sync.dma_start(out_t[:, r, :], o2[:])

    for r in range(NT):
        z_t = z_pool.tile([P, N], BF16, tag=f"z{r % 4}")
        nc.vector.tensor_scalar(z_t[:], v_full[:], u_sb[:, r:r + 1], 1.0,
                                ALU.mult, ALU.max)
        pb = pb_pool.tile([P, N], BF16, tag=f"pb{r % 4}")
        nc.vector.tensor_tensor(pb[:], z_t[:], a_tiles[r], ALU.mult)

        pbT = pbt_pool.tile([P, N], BF16, tag=f"pt{r % 4}")
        o_ps = ops_pool.tile([P, 132], F32, tag=f"o{r % 2}")
        for g in range(2):
            tp = tps_pool.tile([P, 8 * P], BF16, tag=f"tp{(2 * r + g) % 3}")
            for c8 in range(8):
                c = 8 * g + c8
                nc.tensor.matmul(tp[:, c8 * P:(c8 + 1) * P],
                                 pb[:, c * P:(c + 1) * P], ident[:],
                                 is_transpose=True)
            # PSUM->SBUF copies: all on ACT, bitcast to f32 so the
            # per-element engine cost covers two bf16 values per cycle
            nc.scalar.copy(pbT[:, g * 1024:(g + 1) * 1024].bitcast(F32),
                           tp[:].bitcast(F32))
            for c8 in range(8):
                c = 8 * g + c8
                nc.tensor.matmul(o_ps[:, 0:D + 1], pbT[:, c * P:(c + 1) * P],
                                 hp_aug[:, c, 0:D + 1],
                                 start=(c == 0), stop=(c == NT - 1))

        pending.append((o_ps, r))
        if len(pending) > 2:
            finalize(*pending.pop(0))

    for item in pending:
        finalize(*item)


_CACHE = {}


def _get_compiled():
    if "nc" in _CACHE:
        return _CACHE["nc"], _CACHE["names"]

    nc = bacc.Bacc("TRN2", target_bir_lowering=False, debug=False)
    io = {}
    io["a"] = nc.dram_tensor("a", [N, N], F32, kind="ExternalInput").ap()
    io["h"] = nc.dram_tensor("h", [D, N], F32, kind="ExternalInput").ap()
    io["params"] = nc.dram_tensor("params", [P, 261], F32, kind="ExternalInput").ap()
    io["out"] = nc.dram_tensor("out", [N, D], BF16, kind="ExternalOutput").ap()

    with tile.TileContext(nc) as tc:
        with ExitStack() as ctx:
            _build_kernel(ctx, tc, io)
    nc.compile()

    _CACHE["nc"] = nc
    _CACHE["names"] = list(io.keys())
    return nc, _CACHE["names"]


def _make_in_maps(a, h, W_proj, b_proj, w_att, b_att):
    a = np.ascontiguousarray(a, dtype=np.float32)
    h = np.ascontiguousarray(h, dtype=np.float32)
    W_proj = np.ascontiguousarray(W_proj, dtype=np.float32)
    b_proj = np.ascontiguousarray(b_proj, dtype=np.float32).reshape(D, 1)
    w_att = np.ascontiguousarray(w_att, dtype=np.float32)
    w_sd = np.stack([w_att[:D], w_att[D:]], axis=1).copy()  # [D, 2]
    b_att = np.asarray(b_att, dtype=np.float32).reshape(1, 1).copy()

    params = np.zeros((P, 261), dtype=np.float32)
    params[:, 0:D] = W_proj
    params[:, D:2 * D] = np.eye(P, dtype=np.float32)
    params[:, 256:257] = b_proj
    params[:, 257] = W_proj @ w_att[:D]
    params[:, 258] = W_proj @ w_att[D:]
    params[:, 259] = float(b_proj[:, 0] @ w_att[:D])
    params[0, 260] = float(b_proj[:, 0] @ w_att[D:]) + b_att[0, 0]

    in_maps = []
    for c in range(N_CORES):
        in_maps.append({"a": a[c],
                        "h": np.ascontiguousarray(h[c].T),
                        "params": params})
    return in_maps


def _get_executable():
    """Build (once) a sharded PJRT callable for the compiled Bass module.

    Mirrors concourse.bass2jax.run_bass_via_pjrt but keeps the jitted
    function so repeated calls don't retrace/recompile.
    """
    if "exe" in _CACHE:
        return _CACHE["exe"]

    import jax
    from jax.sharding import Mesh, PartitionSpec
    from jax.experimental.shard_map import shard_map
    from concourse import bass2jax, mybir as _mybir

    nc, _ = _get_compiled()
    bass2jax.install_neuronx_cc_hook()

    partition_name = (nc.partition_id_tensor.name
                      if nc.partition_id_tensor else None)
    in_names, out_names, out_avals, zero_outs = [], [], [], []
    for alloc in nc.m.functions[0].allocations:
        if not isinstance(alloc, _mybir.MemoryLocationSet):
            continue
        name = alloc.memorylocations[0].name
        if alloc.kind == "ExternalInput":
            if name != partition_name:
                in_names.append(name)
        elif alloc.kind == "ExternalOutput":
            shape = tuple(alloc.tensor_shape)
            dtype = _mybir.dt.np(alloc.dtype)
            out_names.append(name)
            out_avals.append(jax.core.ShapedArray(shape, dtype))
            zero_outs.append(np.zeros(shape, dtype))
    n_params = len(in_names)
    n_outs = len(out_avals)
    all_in_names = in_names + out_names + (
        [partition_name] if partition_name else [])
    donate = tuple(range(n_params, n_params + n_outs))

    def _body(*args):
        operands = list(args)
        if partition_name is not None:
            operands.append(bass2jax.partition_id_tensor())
        outs = bass2jax._bass_exec_p.bind(
            *operands,
            out_avals=tuple(out_avals),
            in_names=tuple(all_in_names),
            out_names=tuple(out_names),
            lowering_input_output_aliases=(),
            sim_require_finite=True,
            sim_require_nnan=True,
            nc=nc,
        )
        return tuple(outs)

    devices = jax.devices()[:N_CORES]
    mesh = Mesh(np.asarray(devices), ("core",))
    in_specs = (PartitionSpec("core"),) * (n_params + n_outs)
    out_specs = (PartitionSpec("core"),) * n_outs
    fn = jax.jit(
        shard_map(_body, mesh=mesh, in_specs=in_specs, out_specs=out_specs,
                  check_rep=False),
        donate_argnums=donate, keep_unused=True,
    )
    exe = {
        "fn": fn, "mesh": mesh, "in_names": in_names,
        "out_names": out_names, "out_avals": out_avals,
        "zero_outs": zero_outs, "n_params": n_params,
    }
    _CACHE["exe"] = exe
    return exe


def _concat_inputs(exe, in_maps):
    return [
        np.concatenate([np.asarray(in_maps[c][name])
                        for c in range(N_CORES)], axis=0)
        for name in exe["in_names"]
    ]


def _concat_zeros(exe):
    return [np.zeros((N_CORES * z.shape[0], *z.shape[1:]), z.dtype)
            for z in exe["zero_outs"]]


def kernel(a, h, W_proj, b_proj, w_att, b_att):
    exe = _get_executable()
    in_maps = _make_in_maps(a, h, W_proj, b_proj, w_att, b_att)
    out_arrs = exe["fn"](*_concat_inputs(exe, in_maps), *_concat_zeros(exe))
    i = exe["out_names"].index("out")
    return np.asarray(out_arrs[i]).astype(np.float32).reshape(N_CORES, N, D)


if __name__ == "__main__":
    rng = np.random.default_rng(0)
    a = rng.random((B, N, N), dtype=np.float32)
    h = rng.standard_normal((B, N, D)).astype(np.float32)
    W_proj = (rng.standard_normal((D, D)) / np.sqrt(D)).astype(np.float32)
    b_proj = (rng.standard_normal(D) * 0.01).astype(np.float32)
    w_att = (rng.standard_normal(2 * D) / np.sqrt(2 * D)).astype(np.float32)
    b_att = np.float32(rng.standard_normal() * 0.01)

    got = kernel(a=a, h=h, W_proj=W_proj, b_proj=b_proj, w_att=w_att,
                 b_att=b_att)

    hp = h @ W_proj + b_proj
    s = hp @ w_att[:D]
    t = hp @ w_att[D:]
    e = np.maximum(s[:, :, None] + t[:, None, :] + b_att, 0.0)
    att = np.exp(e) * a
    att = att / att.sum(-1, keepdims=True)
    ref = att @ hp + hp

    err = np.abs(got - ref).max() / np.abs(ref).max()
    print("rel err:", err)
